# revision 6
# baseline (speedup 1.0000x reference)
"""DiffuseRouter kernel for 8 TRN2 NeuronCores.

Reference computation (enable_time=False, soft_time_routing=True):
    out[b, l, d] = (1/3) * sum_g sum_e expert_emb_g[e, b, l, d]
i.e. a uniform-weighted sum of 28 expert planes per batch element.

Sharding: pure data-parallel over batch B=8 -> one batch element per core.
Each core reads its 28 [256, 1280] f32 planes (36.7 MB), reduces them
on-chip, scales by 1/3, and writes its [256, 1280] output.  No collectives.

v9 = engine-balanced loads via PE routing matrices.
Trace analysis of v8 showed SDMA engine 15 (partitions 92-95/124-127 per
the confirmed port swizzle port=((p>>2)&7)<<1|((p>>6)&1)) running ~21.4
GB/s vs 25.6 on the other 15 engines; uniform [128, N] loads made it the
critical path (109 us busy) while the rest idled 21 us.  v9 gives the 8
slow-port partitions only 24 of their 28 plane-chunks; the missing 32
chunks (planes 24-27 of the 8 slow rows) are re-homed onto partitions
0-31 as a 29th slot and summed into the right output rows by one extra
0/1 routing matmul (lhsT.T@rhs contracts over partitions, so a [32,128]
0/1 matrix routes partition q's chunk into psum row slowrow(q)).  All
columns now go through the PE: 5 PSUM banks of 512 f32 accumulate 29
slot-matmuls each; ACT scales x1/3 out of PSUM and stores.

Loads are quad-packed: each partition's stream is host-packed contiguous,
so a [range, 4*2560] f32 load emits one 40,960 B descriptor per partition
(~1.8% fixed-cost overhead vs 3.5% for v8's 20 KB pairs).  Per-port bytes:
slow port 15 = 1.97 MB, even fast ports = 2.34 MB, odd fast ports = 2.29
MB -> ~92 us stream regardless of whether engine 15 is in its slow mode.
"""

import numpy as np

import concourse.bacc as bacc
import concourse.tile as tile
from concourse import mybir
from concourse.bass_utils import run_bass_kernel_spmd

N_CORES = 8
E_TOTAL = 28  # 4 + 8 + 16 experts across the 3 granularity levels
L, D = 256, 1280
P = 128  # SBUF partitions
FD = (L // P) * D  # 2560 free-dim elements per partition per plane
BW = 512  # one 2 KB PSUM bank of f32
NW = FD // BW  # 5 psum windows
SCALE = 1.0 / 3.0

# Slow SDMA port 15 serves partitions 92-95 and 124-127 (AWS port table).
SLOW_ROWS = (92, 93, 94, 95, 124, 125, 126, 127)
N_SLOW = 24  # plane-chunks kept on each slow partition (planes 0..23)
N_QUAD = 7  # 28 slots loaded in groups of 4

_NC_CACHE = None


def _build_nc():
    """Build the SPMD Bass program (identical on all 8 cores)."""
    nc = bacc.Bacc(
        "TRN2", target_bir_lowering=False, debug=False, enable_partition_id=False
    )
    f32 = mybir.dt.float32
    f32r = mybir.dt.float32r

    # Per-partition contiguous streams, grouped by uniform stream length:
    #   xa: partitions  0-31, 29 slots (28 own planes + 1 foreign chunk)
    #   xb: partitions 32-91, 28 slots
    #   xc: partitions 96-123, 28 slots
    #   xd: partitions 92-95, 24 slots   (slow port)
    #   xe: partitions 124-127, 24 slots (slow port)
    xa = nc.dram_tensor("xa", [32, 29 * FD], f32, kind="ExternalInput")
    xb = nc.dram_tensor("xb", [60, 28 * FD], f32, kind="ExternalInput")
    xc = nc.dram_tensor("xc", [28, 28 * FD], f32, kind="ExternalInput")
    xd = nc.dram_tensor("xd", [4, 24 * FD], f32, kind="ExternalInput")
    xe = nc.dram_tensor("xe", [4, 24 * FD], f32, kind="ExternalInput")
    ident_d = nc.dram_tensor("ident", [P, P], f32, kind="ExternalInput")
    identm_d = nc.dram_tensor("identm", [P, P], f32, kind="ExternalInput")
    m28_d = nc.dram_tensor("m28", [32, P], f32, kind="ExternalInput")
    out = nc.dram_tensor("out", [P, FD], f32, kind="ExternalOutput")

    xa_r = xa.ap().bitcast(f32r)
    xb_r = xb.ap().bitcast(f32r)
    xc_r = xc.ap().bitcast(f32r)
    xd_r = xd.ap().bitcast(f32r)
    xe_r = xe.ap().bitcast(f32r)

    with tile.TileContext(nc) as tc:
        with (
            tc.tile_pool(name="in", bufs=4) as pin,
            tc.tile_pool(name="one", bufs=1) as pone,
            tc.tile_pool(name="const", bufs=1) as pconst,
            tc.tile_pool(name="acc", bufs=1) as pacc,
            tc.tile_pool(name="ps", bufs=1, space="PSUM") as pps,
        ):
            ident = pconst.tile([P, P], f32r, name="ident", tag="ident")
            identm = pconst.tile([P, P], f32r, name="identm", tag="identm")
            m28 = pconst.tile([32, P], f32r, name="m28", tag="m28")
            # Weights ride the ACT ring so the sync ring carries only loads.
            nc.scalar.dma_start(out=ident[:], in_=ident_d.ap().bitcast(f32r))
            nc.scalar.dma_start(out=identm[:], in_=identm_d.ap().bitcast(f32r))
            nc.scalar.dma_start(out=m28[:], in_=m28_d.ap().bitcast(f32r))

            psums = [
                pps.tile([P, BW], f32, name=f"ps{w}", tag=f"ps{w}")
                for w in range(NW)
            ]
            outs = pacc.tile([P, FD], f32, name="outs", tag="outs")
            t28 = pone.tile([32, FD], f32r, name="t28", tag="t28")

            for q in range(N_QUAD):
                t = pin.tile([P, 4 * FD], f32r)
                c0, c1 = q * 4 * FD, (q + 1) * 4 * FD
                nc.sync.dma_start(out=t[0:32, :], in_=xa_r[:, c0:c1])
                nc.sync.dma_start(out=t[32:92, :], in_=xb_r[:, c0:c1])
                nc.sync.dma_start(out=t[96:124, :], in_=xc_r[:, c0:c1])
                if q < 6:
                    nc.sync.dma_start(out=t[92:96, :], in_=xd_r[:, c0:c1])
                    nc.sync.dma_start(out=t[124:128, :], in_=xe_r[:, c0:c1])
                if q == 0:
                    # Foreign chunks (planes 24-27 of the slow rows), slot 28.
                    nc.sync.dma_start(
                        out=t28[:], in_=xa_r[:, 28 * FD : 29 * FD]
                    )
                for j in range(4):
                    s = 4 * q + j
                    for w in range(NW):
                        cs = slice(j * FD + w * BW, j * FD + (w + 1) * BW)
                        # Slots >= 24: slow partitions hold stale (but
                        # real-float) data from an earlier quad; identm has
                        # their identity rows zeroed so it contributes 0.
                        wt = ident if s < N_SLOW else identm
                        nc.tensor.matmul(
                            psums[w][:], wt[:], t[:, cs],
                            start=(s == 0), stop=False,
                        )

            # Slot 28: route foreign chunks into their slow output rows,
            # then drain PSUM through ACT (x1/3) and store.
            for w in range(NW):
                ws = slice(w * BW, (w + 1) * BW)
                nc.tensor.matmul(
                    psums[w][:], m28[:], t28[:, ws], start=False, stop=True
                )
                nc.scalar.mul(outs[:, ws], psums[w][:], SCALE)
                nc.scalar.dma_start(out=out.ap()[:, ws], in_=outs[:, ws])
    nc.compile()
    return nc


def _get_nc():
    global _NC_CACHE
    if _NC_CACHE is None:
        _NC_CACHE = _build_nc()
    return _NC_CACHE


def _pack_core(v):
    """v: [28, 128, 2560] planes for one batch element -> input map."""
    w = np.transpose(v, (1, 0, 2))  # [128 partitions, 28 planes, 2560]
    xa = np.empty((32, 29 * FD), dtype=np.float32)
    xa[:, : 28 * FD] = w[0:32].reshape(32, 28 * FD)
    for q in range(32):
        # Foreign chunk: plane 24 + q%4 of slow row SLOW_ROWS[q//4].
        xa[q, 28 * FD :] = v[24 + (q % 4), SLOW_ROWS[q // 4]]
    m28 = np.zeros((32, P), dtype=np.float32)
    for q in range(32):
        m28[q, SLOW_ROWS[q // 4]] = 1.0
    identm = np.eye(P, dtype=np.float32)
    for r in SLOW_ROWS:
        identm[r, r] = 0.0
    return {
        "xa": xa,
        "identm": identm,
        "xb": np.ascontiguousarray(w[32:92].reshape(60, 28 * FD)),
        "xc": np.ascontiguousarray(w[96:124].reshape(28, 28 * FD)),
        "xd": np.ascontiguousarray(w[92:96, :N_SLOW].reshape(4, 24 * FD)),
        "xe": np.ascontiguousarray(w[124:128, :N_SLOW].reshape(4, 24 * FD)),
        "ident": np.eye(P, dtype=np.float32),
        "m28": m28,
    }


def _run(inputs, trace=False, trace_kwargs=None):
    e0 = np.asarray(inputs["expert_emb_0"], dtype=np.float32)
    e1 = np.asarray(inputs["expert_emb_1"], dtype=np.float32)
    e2 = np.asarray(inputs["expert_emb_2"], dtype=np.float32)
    B = e0.shape[1]
    assert B == N_CORES, f"expected B == {N_CORES}, got {B}"

    in_maps = []
    for b in range(B):
        xb_full = np.concatenate([e0[:, b], e1[:, b], e2[:, b]], axis=0)
        v = xb_full.reshape(E_TOTAL, P, FD)
        in_maps.append(_pack_core(v))

    kw = {}
    if trace:
        kw["trace"] = True
        if trace_kwargs:
            kw.update(trace_kwargs)
    try:
        res = run_bass_kernel_spmd(_get_nc(), in_maps, list(range(N_CORES)), **kw)
    except Exception:
        # One retry: transient device errors usually clear on re-dispatch.
        res = run_bass_kernel_spmd(_get_nc(), in_maps, list(range(N_CORES)), **kw)
    out = np.stack(
        [res.results[b]["out"].reshape(L, D) for b in range(B)], axis=0
    )
    return out.astype(np.float32, copy=False), res


def kernel(**inputs) -> np.ndarray:
    out, _ = _run(inputs, trace=False)
    return out


# revision 7
# speedup vs baseline: 1.9600x; 1.9600x over previous
"""DiffuseRouter kernel for 8 TRN2 NeuronCores.

Reference computation (enable_time=False, soft_time_routing=True):
    out[b, l, d] = (1/3) * sum_g sum_e expert_emb_g[e, b, l, d]
i.e. a uniform-weighted sum of 28 expert planes per batch element.

Sharding: pure data-parallel over batch B=8 -> one batch element per core.
Each core reads its 28 [256, 1280] f32 planes (36.7 MB), reduces them
on-chip, scales by 1/3, and writes its [256, 1280] output.  No collectives.

v10 = engine-balanced loads.  Trace facts this build is shaped around:
  * HWDGE descriptors of one dma_start are dealt to the 16 SDMA engines in
    contiguous chunks of ceil(n_desc/16) starting at engine 0; trailing
    engines get nothing.  A [120, N] transfer therefore never touches
    engine 15, and a [128, N] transfer maps partitions 8k..8k+7 -> engine k.
  * SDMA engine 15 intermittently runs ~21.4 GB/s vs 25.6 on engines 0-14
    (v8 trace: 109 us busy on engine 15 while the rest idled 21 us).
  * Only dma_starts whose DRAM side is one contiguous block run at line
    rate; strided sources measured 12 GB/s (v9 post-mortem).
Plan: partitions 120-127 keep only planes 0-23 (the 12 uniform pair
loads); their planes 24-27 are re-homed onto partitions 0-31 and routed
into the right PSUM rows by a [32,128] 0/1 matmul (lhsT.T@rhs contracts
over partitions).  Planes 24-27 of partitions 0-119 load as two [120, 2FD]
tail transfers and reduce with K=120 matmuls.  Engine 15 carries exactly
the 12 pair loads (1.97 MB -> ~92 us at its slow rate); engines 0-14 carry
~2.41 MB (~94 us) -> balanced at the ~405 GB/s aggregate roofline.

All columns go through the PE: 5 PSUM banks of 512 f32 accumulate 29
slot-matmuls each (fp32r identity / routing weights, 1 cycle/row); ACT
scales x1/3 out of PSUM; stores are engine-15-free ([0:120)+[120:128)
splits into per-window contiguous DRAM tensors).
"""

import numpy as np

import concourse.bacc as bacc
import concourse.tile as tile
from concourse import mybir
from concourse.bass_utils import run_bass_kernel_spmd

N_CORES = 8
E_TOTAL = 28  # 4 + 8 + 16 experts across the 3 granularity levels
L, D = 256, 1280
P = 128  # SBUF partitions
FD = (L // P) * D  # 2560 free-dim elements per partition per plane
BW = 512  # one 2 KB PSUM bank of f32
NW = FD // BW  # 5 psum windows
SCALE = 1.0 / 3.0
NF = 120  # fast partitions 0-119; 120-127 ride slow engine 15 only in pairs
N_PAIR = 12  # planes 0-23 as uniform [128, 2*FD] pair loads

_NC_CACHE = None


def _build_nc():
    """Build the SPMD Bass program (identical on all 8 cores)."""
    nc = bacc.Bacc(
        "TRN2", target_bir_lowering=False, debug=False, enable_partition_id=False
    )
    f32 = mybir.dt.float32
    f32r = mybir.dt.float32r

    # All DRAM blocks are contiguous per dma_start (v9 post-mortem).
    xp = nc.dram_tensor("xp", [N_PAIR, P, 2 * FD], f32, kind="ExternalInput")
    xta = nc.dram_tensor("xta", [NF, 2 * FD], f32, kind="ExternalInput")
    xtb = nc.dram_tensor("xtb", [NF, 2 * FD], f32, kind="ExternalInput")
    xf = nc.dram_tensor("xf", [32, FD], f32, kind="ExternalInput")
    ident_d = nc.dram_tensor("ident", [P, P], f32, kind="ExternalInput")
    m28_d = nc.dram_tensor("m28", [32, P], f32, kind="ExternalInput")
    outs_d = [
        nc.dram_tensor(f"out{w}", [P, BW], f32, kind="ExternalOutput")
        for w in range(NW)
    ]

    xp_r = xp.ap().bitcast(f32r)
    xta_r = xta.ap().bitcast(f32r)
    xtb_r = xtb.ap().bitcast(f32r)
    xf_r = xf.ap().bitcast(f32r)

    with tile.TileContext(nc) as tc:
        with (
            tc.tile_pool(name="in", bufs=5) as pin,
            tc.tile_pool(name="tail", bufs=1) as ptail,
            tc.tile_pool(name="one", bufs=1) as pone,
            tc.tile_pool(name="const", bufs=1) as pconst,
            tc.tile_pool(name="acc", bufs=1) as pacc,
            tc.tile_pool(name="ps", bufs=1, space="PSUM") as pps,
        ):
            ident = pconst.tile([P, P], f32r, name="ident", tag="ident")
            m28 = pconst.tile([32, P], f32r, name="m28", tag="m28")
            # Weights ride the ACT ring so the sync ring carries only loads.
            nc.scalar.dma_start(out=ident[:], in_=ident_d.ap().bitcast(f32r))
            nc.scalar.dma_start(out=m28[:], in_=m28_d.ap().bitcast(f32r))

            psums = [
                pps.tile([P, BW], f32, name=f"ps{w}", tag=f"ps{w}")
                for w in range(NW)
            ]
            outs = pacc.tile([P, FD], f32, name="outs", tag="outs")
            t28 = pone.tile([32, FD], f32r, name="t28", tag="t28")

            def consume(s, t, col0, wt, kslice):
                """Accumulate slot s (tile cols col0..col0+FD) into PSUM."""
                for w in range(NW):
                    cs = slice(col0 + w * BW, col0 + (w + 1) * BW)
                    nc.tensor.matmul(
                        psums[w][:], wt, t[kslice, cs],
                        start=(s == 0), stop=False,
                    )

            # Planes 0-23: uniform pair loads; engine k <- partitions 8k..8k+7.
            for g in range(N_PAIR):
                t = pin.tile([P, 2 * FD], f32r)
                nc.sync.dma_start(out=t[:], in_=xp_r[g])
                if g == 0:
                    # Foreign chunks (planes 24-27 of rows 120-127) park in
                    # SBUF early; split 30+2 keeps engine 15 idle.
                    nc.sync.dma_start(out=t28[0:30, :], in_=xf_r[0:30, :])
                    nc.sync.dma_start(out=t28[30:32, :], in_=xf_r[30:32, :])
                consume(2 * g, t, 0, ident[:], slice(0, P))
                consume(2 * g + 1, t, FD, ident[:], slice(0, P))

            # Planes 24-27 of partitions 0-119: [120, N] loads skip engine 15.
            for i, src in enumerate((xta_r, xtb_r)):
                tt = ptail.tile([NF, 2 * FD], f32r, name=f"tt{i}", tag=f"tt{i}")
                nc.sync.dma_start(out=tt[:], in_=src)
                consume(24 + 2 * i, tt, 0, ident[0:NF, :], slice(0, NF))
                consume(25 + 2 * i, tt, FD, ident[0:NF, :], slice(0, NF))

            # Slot 28: route foreign chunks into rows 120-127, then drain
            # PSUM through ACT (x1/3) and store engine-15-free.
            for w in range(NW):
                ws = slice(w * BW, (w + 1) * BW)
                nc.tensor.matmul(
                    psums[w][:], m28[:], t28[:, ws], start=False, stop=True
                )
                nc.scalar.mul(outs[:, ws], psums[w][:], SCALE)
                nc.scalar.dma_start(
                    out=outs_d[w].ap()[0:NF, :], in_=outs[0:NF, ws]
                )
                nc.scalar.dma_start(
                    out=outs_d[w].ap()[NF:P, :], in_=outs[NF:P, ws]
                )
    nc.compile()
    return nc


def _get_nc():
    global _NC_CACHE
    if _NC_CACHE is None:
        _NC_CACHE = _build_nc()
    return _NC_CACHE


def _pack_core(v):
    """v: [28, 128, 2560] planes for one batch element -> input map."""
    xp = (
        v[:24]
        .reshape(N_PAIR, 2, P, FD)
        .transpose(0, 2, 1, 3)
        .reshape(N_PAIR, P, 2 * FD)
    )
    xta = v[24:26, :NF].transpose(1, 0, 2).reshape(NF, 2 * FD)
    xtb = v[26:28, :NF].transpose(1, 0, 2).reshape(NF, 2 * FD)
    xf = np.empty((32, FD), dtype=np.float32)
    for q in range(32):
        xf[q] = v[24 + (q % 4), NF + (q // 4)]
    m28 = np.zeros((32, P), dtype=np.float32)
    for q in range(32):
        m28[q, NF + (q // 4)] = 1.0
    return {
        "xp": np.ascontiguousarray(xp),
        "xta": np.ascontiguousarray(xta),
        "xtb": np.ascontiguousarray(xtb),
        "xf": xf,
        "ident": np.eye(P, dtype=np.float32),
        "m28": m28,
    }


def _run(inputs, trace=False, trace_kwargs=None):
    e0 = np.asarray(inputs["expert_emb_0"], dtype=np.float32)
    e1 = np.asarray(inputs["expert_emb_1"], dtype=np.float32)
    e2 = np.asarray(inputs["expert_emb_2"], dtype=np.float32)
    B = e0.shape[1]
    assert B == N_CORES, f"expected B == {N_CORES}, got {B}"

    in_maps = []
    for b in range(B):
        xb_full = np.concatenate([e0[:, b], e1[:, b], e2[:, b]], axis=0)
        v = xb_full.reshape(E_TOTAL, P, FD)
        in_maps.append(_pack_core(v))

    kw = {}
    if trace:
        kw["trace"] = True
        if trace_kwargs:
            kw.update(trace_kwargs)
    try:
        res = run_bass_kernel_spmd(_get_nc(), in_maps, list(range(N_CORES)), **kw)
    except Exception:
        # One retry: transient device errors usually clear on re-dispatch.
        res = run_bass_kernel_spmd(_get_nc(), in_maps, list(range(N_CORES)), **kw)
    out = np.stack(
        [
            np.concatenate(
                [res.results[b][f"out{w}"] for w in range(NW)], axis=1
            ).reshape(L, D)
            for b in range(B)
        ],
        axis=0,
    )
    return out.astype(np.float32, copy=False), res


def kernel(**inputs) -> np.ndarray:
    out, _ = _run(inputs, trace=False)
    return out


# revision 13
# speedup vs baseline: 1.9864x; 1.0135x over previous
"""DiffuseRouter kernel for 8 TRN2 NeuronCores.

Reference computation (enable_time=False, soft_time_routing=True):
    out[b, l, d] = (1/3) * sum_g sum_e expert_emb_g[e, b, l, d]
i.e. a uniform-weighted sum of 28 expert planes per batch element.

Sharding: pure data-parallel over batch B=8 -> one batch element per core.
Each core reads its 28 [256, 1280] f32 planes (36.7 MB), reduces them
on-chip, scales by 1/3, and writes its [256, 1280] output.  No collectives.

v11 = engine-balanced loads + hybrid PE/DVE compute.  Trace facts this
build is shaped around:
  * HWDGE descriptors of one dma_start are dealt to the 16 SDMA engines in
    contiguous chunks of ceil(n_desc/16) starting at engine 0; trailing
    engines get nothing.  A [120, N] transfer therefore never touches
    engine 15, and a [128, N] transfer maps partitions 8k..8k+7 -> engine k.
  * SDMA engine 15 intermittently runs ~21.4 GB/s vs 25.6 on engines 0-14
    (v8 trace: 109 us busy on engine 15 while the rest idled 21 us).
  * Only dma_starts whose DRAM side is one contiguous block run at line
    rate; strided sources measured 12 GB/s (v9 post-mortem).
  * An all-PE reduction (145 matmuls, tensor-engine 44% active) trips the
    activity throttle (util limit 0.5 for 53% of the run) and drags every
    SDMA engine to ~22.3 GB/s (v10 post-mortem: 130.7 us).  PE work must
    stay near v8's ~30% active -> 3 PSUM banks + DVE for the rest.
Plan: partitions 120-127 keep only planes 0-23 (the 12 uniform pair
loads); their planes 24-27 are re-homed onto partitions 0-31 and routed
into the right PSUM rows by a [32,128] 0/1 matmul (lhsT.T@rhs contracts
over partitions, and also sums the 4 planes per row since 4 partitions
map to one output row).  Planes 24-27 of partitions 0-119 load as two
[120, 2FD] tail transfers (K=120 matmuls / [0:120) DVE ops).  Engine 15
carries exactly the 12 pair loads (1.97 MB -> ~92 us at its slow rate);
engines 0-14 carry ~2.41 MB (~94 us) -> balanced at the ~405 GB/s
aggregate roofline.

Compute: PE accumulates cols [0:1536) in 3 PSUM banks (fp32r identity /
routing weights); DVE accumulates cols [1536:2560) unscaled in SBUF acc.
Final: two matmuls per DVE window (ident @ acc + m28 @ t28) land them in
PSUM banks 3-4; ACT scales everything x1/3 out of PSUM; stores are
engine-15-free ([0:120)+[120:128) splits into per-window contiguous DRAM
tensors).
"""

import numpy as np

import concourse.bacc as bacc
import concourse.tile as tile
from concourse import mybir
from concourse.alu_op_type import AluOpType
from concourse.bass_utils import run_bass_kernel_spmd

N_CORES = 8
E_TOTAL = 28  # 4 + 8 + 16 experts across the 3 granularity levels
L, D = 256, 1280
P = 128  # SBUF partitions
FD = (L // P) * D  # 2560 free-dim elements per partition per plane
BW = 512  # one 2 KB PSUM bank of f32
NW = FD // BW  # 5 psum windows
NB_PE = 3  # windows summed on TensorE during the stream (cols 0..1536)
DVE_LO = NB_PE * BW  # 1536: start of the DVE column range
DVE_W = FD - DVE_LO  # 1024 cols accumulated on DVE
SCALE = 1.0 / 3.0
NF = 120  # fast partitions 0-119; 120-127 ride slow engine 15 only in pairs
N_PAIR = 12  # planes 0-23 as uniform [128, 2*FD] pair loads

_NC_CACHE = None


def _build_nc():
    """Build the SPMD Bass program (identical on all 8 cores)."""
    nc = bacc.Bacc(
        "TRN2", target_bir_lowering=False, debug=False, enable_partition_id=False
    )
    f32 = mybir.dt.float32
    f32r = mybir.dt.float32r

    # All DRAM blocks are contiguous per dma_start (v9 post-mortem).
    xp = nc.dram_tensor("xp", [N_PAIR, P, 2 * FD], f32, kind="ExternalInput")
    xta = nc.dram_tensor("xta", [NF, 2 * FD], f32, kind="ExternalInput")
    xtb = nc.dram_tensor("xtb", [NF, 2 * FD], f32, kind="ExternalInput")
    xf = nc.dram_tensor("xf", [32, FD], f32, kind="ExternalInput")
    ident_d = nc.dram_tensor("ident", [P, P], f32, kind="ExternalInput")
    m28_d = nc.dram_tensor("m28", [32, P], f32, kind="ExternalInput")
    outs_d = [
        nc.dram_tensor(f"out{w}", [P, BW], f32, kind="ExternalOutput")
        for w in range(NW)
    ]

    xp_r = xp.ap().bitcast(f32r)
    xta_r = xta.ap().bitcast(f32r)
    xtb_r = xtb.ap().bitcast(f32r)
    xf_r = xf.ap().bitcast(f32r)

    with tile.TileContext(nc) as tc:
        with (
            tc.tile_pool(name="in", bufs=5) as pin,
            tc.tile_pool(name="tail", bufs=1) as ptail,
            tc.tile_pool(name="one", bufs=1) as pone,
            tc.tile_pool(name="const", bufs=1) as pconst,
            tc.tile_pool(name="acc", bufs=1) as pacc,
            tc.tile_pool(name="ps", bufs=1, space="PSUM") as pps,
        ):
            ident = pconst.tile([P, P], f32r, name="ident", tag="ident")
            m28 = pconst.tile([32, P], f32r, name="m28", tag="m28")
            # Weights ride the ACT ring so the sync ring carries only loads.
            nc.scalar.dma_start(out=ident[:], in_=ident_d.ap().bitcast(f32r))
            nc.scalar.dma_start(out=m28[:], in_=m28_d.ap().bitcast(f32r))

            psums = [
                pps.tile([P, BW], f32, name=f"ps{w}", tag=f"ps{w}")
                for w in range(NW)
            ]
            outs = pacc.tile([P, FD], f32, name="outs", tag="outs")
            acc = pacc.tile([P, DVE_W], f32, name="acc", tag="acc")
            t28 = pone.tile([32, FD], f32r, name="t28", tag="t28")

            mult = AluOpType.mult
            add = AluOpType.add

            def consume(s, t, col0, wt, kslice):
                """Accumulate slot s (tile cols col0..col0+FD): PE takes
                cols [0:1536), DVE accumulates [1536:2560) unscaled."""
                for w in range(NB_PE):
                    cs = slice(col0 + w * BW, col0 + (w + 1) * BW)
                    nc.tensor.matmul(
                        psums[w][:], wt, t[kslice, cs],
                        start=(s == 0), stop=False,
                    )
                src = t[kslice, col0 + DVE_LO : col0 + FD].bitcast(f32)
                if s == 0:
                    nc.vector.tensor_scalar_mul(acc[kslice, :], src, 1.0)
                else:
                    nc.vector.scalar_tensor_tensor(
                        acc[kslice, :], src, 1.0, acc[kslice, :], mult, add
                    )

            # Planes 0-23: uniform pair loads; engine k <- partitions 8k..8k+7.
            for g in range(N_PAIR):
                t = pin.tile([P, 2 * FD], f32r)
                nc.sync.dma_start(out=t[:], in_=xp_r[g])
                if g == 0:
                    # Foreign chunks (planes 24-27 of rows 120-127) park in
                    # SBUF early; split 30+2 keeps engine 15 idle.
                    nc.sync.dma_start(out=t28[0:30, :], in_=xf_r[0:30, :])
                    nc.sync.dma_start(out=t28[30:32, :], in_=xf_r[30:32, :])
                consume(2 * g, t, 0, ident[:], slice(0, P))
                consume(2 * g + 1, t, FD, ident[:], slice(0, P))

            # Planes 24-27 of partitions 0-119: [120, N] loads skip engine 15.
            for i, src in enumerate((xta_r, xtb_r)):
                tt = ptail.tile([NF, 2 * FD], f32r, name=f"tt{i}", tag=f"tt{i}")
                nc.sync.dma_start(out=tt[:], in_=src)
                consume(24 + 2 * i, tt, 0, ident[0:NF, :], slice(0, NF))
                consume(25 + 2 * i, tt, FD, ident[0:NF, :], slice(0, NF))

            # Slot 28 (PE zone): route foreign chunks into rows 120-127.
            for w in range(NW):
                ws = slice(w * BW, (w + 1) * BW)
                if w < NB_PE:
                    nc.tensor.matmul(
                        psums[w][:], m28[:], t28[:, ws], start=False, stop=True
                    )
                else:
                    # DVE windows: land SBUF acc + routed foreign in PSUM.
                    # acc is DVE-produced f32 (not fp32r-rounded), so these
                    # two matmuls run in plain fp32 -- ~1 us at the tail.
                    asl = slice((w - NB_PE) * BW, (w - NB_PE + 1) * BW)
                    nc.tensor.matmul(
                        psums[w][:], ident[:].bitcast(f32), acc[:, asl],
                        start=True, stop=False,
                    )
                    nc.tensor.matmul(
                        psums[w][:], m28[:].bitcast(f32),
                        t28[:, ws].bitcast(f32), start=False, stop=True,
                    )
                nc.scalar.mul(outs[:, ws], psums[w][:], SCALE)
                nc.scalar.dma_start(
                    out=outs_d[w].ap()[0:NF, :], in_=outs[0:NF, ws]
                )
                nc.scalar.dma_start(
                    out=outs_d[w].ap()[NF:P, :], in_=outs[NF:P, ws]
                )
    nc.compile()
    return nc


def _get_nc():
    global _NC_CACHE
    if _NC_CACHE is None:
        _NC_CACHE = _build_nc()
    return _NC_CACHE


def _pack_core(v):
    """v: [28, 128, 2560] planes for one batch element -> input map."""
    xp = (
        v[:24]
        .reshape(N_PAIR, 2, P, FD)
        .transpose(0, 2, 1, 3)
        .reshape(N_PAIR, P, 2 * FD)
    )
    xta = v[24:26, :NF].transpose(1, 0, 2).reshape(NF, 2 * FD)
    xtb = v[26:28, :NF].transpose(1, 0, 2).reshape(NF, 2 * FD)
    xf = np.empty((32, FD), dtype=np.float32)
    for q in range(32):
        xf[q] = v[24 + (q % 4), NF + (q // 4)]
    m28 = np.zeros((32, P), dtype=np.float32)
    for q in range(32):
        m28[q, NF + (q // 4)] = 1.0
    return {
        "xp": np.ascontiguousarray(xp),
        "xta": np.ascontiguousarray(xta),
        "xtb": np.ascontiguousarray(xtb),
        "xf": xf,
        "ident": np.eye(P, dtype=np.float32),
        "m28": m28,
    }


def _run(inputs, trace=False, trace_kwargs=None):
    e0 = np.asarray(inputs["expert_emb_0"], dtype=np.float32)
    e1 = np.asarray(inputs["expert_emb_1"], dtype=np.float32)
    e2 = np.asarray(inputs["expert_emb_2"], dtype=np.float32)
    B = e0.shape[1]
    assert B == N_CORES, f"expected B == {N_CORES}, got {B}"

    in_maps = []
    for b in range(B):
        xb_full = np.concatenate([e0[:, b], e1[:, b], e2[:, b]], axis=0)
        v = xb_full.reshape(E_TOTAL, P, FD)
        in_maps.append(_pack_core(v))

    kw = {}
    if trace:
        kw["trace"] = True
        if trace_kwargs:
            kw.update(trace_kwargs)
    try:
        res = run_bass_kernel_spmd(_get_nc(), in_maps, list(range(N_CORES)), **kw)
    except Exception:
        # One retry: transient device errors usually clear on re-dispatch.
        res = run_bass_kernel_spmd(_get_nc(), in_maps, list(range(N_CORES)), **kw)
    out = np.stack(
        [
            np.concatenate(
                [res.results[b][f"out{w}"] for w in range(NW)], axis=1
            ).reshape(L, D)
            for b in range(B)
        ],
        axis=0,
    )
    return out.astype(np.float32, copy=False), res


def kernel(**inputs) -> np.ndarray:
    out, _ = _run(inputs, trace=False)
    return out
